# revision 1
# baseline (speedup 1.0000x reference)
"""Trainium2 Bass kernel for nn_FLB_Attention_Layer (gated fusion + additive
attention over 3 tokens + output projection, with residuals).

Strategy: pure data-parallel over batch B=4096 across 8 NeuronCores
(512 samples/core, weights replicated). Inside each core:

- All activations are kept FEATURE-MAJOR in SBUF: [128 part = feature%128,
  k-tile = feature//128, token, batch]. Matmuls contract features on the
  partition dim (lhsT = W.T column block, rhs = activations).
- Weights are loaded row-contiguous [128 out-rows, 2048] and transposed
  on-chip via TensorE transpose-mode into W.T column blocks.
- Matmuls run in float32r (fp32 data, ~tf32 accuracy, bf16-rate at N>=512).
- Additive attention per head h (head dim 128 = one partition tile):
  T = tanh(q_i + k_j) built by DVE+ACT (bf16), scores via PE matmuls
  (lhsT = T slice, rhs = v_a column) -> batch-major scores [128b, 9],
  softmax batch-major on DVE/ACT, weights transposed back via PE and
  broadcast across partitions with a constant row-select matmul (SEL),
  weighted sum of v on DVE.
- Attention output (feature-major) roundtrips through a DRAM scratch,
  reusing the token SBUF space for the W_o contraction.
- Residual add done batch-major right before the final store.
"""

import numpy as np

P = 128
D = 2048
H = 16
DH = 128
KT = D // P  # 16 k-tiles
B = 4096
N_CORES = 8
B_C = B // N_CORES  # 512 per core

_compiled = {}


def _build(b_c=B_C, d=D, h=H):
    import concourse.bass as bass
    import concourse.mybir as mybir
    import concourse.tile as tile
    from contextlib import ExitStack
    from concourse import bacc
    from concourse.masks import make_identity

    f32 = mybir.dt.float32
    f32r = mybir.dt.float32r
    bf16 = mybir.dt.bfloat16
    AF = mybir.ActivationFunctionType

    kt = d // P
    nh = h
    nb = b_c // P  # batch tiles

    nc = bacc.Bacc(None, target_bir_lowering=False, debug=False)

    # ---- params ----
    f16 = mybir.dt.float16
    xs = nc.declare_dram_parameter("x", [b_c, d], f16, isOutput=False)
    ls = nc.declare_dram_parameter("lat", [b_c, d], f16, isOutput=False)
    fs = nc.declare_dram_parameter("fdbk", [b_c, d], f16, isOutput=False)
    WgL = nc.declare_dram_parameter("WgL", [d, d], f16, isOutput=False)
    WgX = nc.declare_dram_parameter("WgX", [d, d], f16, isOutput=False)
    Wq = nc.declare_dram_parameter("Wq", [d, d], f16, isOutput=False)
    Wk = nc.declare_dram_parameter("Wk", [d, d], f16, isOutput=False)
    Wv = nc.declare_dram_parameter("Wv", [d, d], f16, isOutput=False)
    Wo = nc.declare_dram_parameter("Wo", [d, d], f16, isOutput=False)
    bgLT = nc.declare_dram_parameter("bgLT", [P, kt], f32, isOutput=False)
    bgXT = nc.declare_dram_parameter("bgXT", [P, kt], f32, isOutput=False)
    vaT = nc.declare_dram_parameter("vaT", [DH, nh], f32, isOutput=False)

    outs = [
        nc.declare_dram_parameter(f"o{t}", [b_c, d], f32, isOutput=True)
        for t in range(3)
    ]
    tok_in = [xs, ls, fs]

    with tile.TileContext(nc) as tc:
        with ExitStack() as ctx:
            const = ctx.enter_context(tc.tile_pool(name="const", bufs=1))
            ptok = ctx.enter_context(tc.tile_pool(name="ptok", bufs=1))
            pbig = ctx.enter_context(tc.tile_pool(name="pbig", bufs=2))
            pwT = ctx.enter_context(tc.tile_pool(name="pwT", bufs=4))
            pact = ctx.enter_context(tc.tile_pool(name="pact", bufs=2))
            pact1 = ctx.enter_context(tc.tile_pool(name="pact1", bufs=2))
            pvh = ctx.enter_context(tc.tile_pool(name="pvh", bufs=2))
            psm = ctx.enter_context(tc.tile_pool(name="psm", bufs=2))
            dram = ctx.enter_context(tc.tile_pool(name="dram", bufs=1, space="DRAM"))
            ps_mm = ctx.enter_context(tc.tile_pool(name="ps_mm", bufs=6, space="PSUM"))
            ps_tp = ctx.enter_context(tc.tile_pool(name="ps_tp", bufs=2, space="PSUM"))

            ident = const.tile([P, P], f32)
            make_identity(nc, ident)
            bgl_t = const.tile([P, kt], f32)
            bgx_t = const.tile([P, kt], f32)
            nc.sync.dma_start(bgl_t[:], bgLT[:])
            nc.sync.dma_start(bgx_t[:], bgXT[:])
            va_f = const.tile([DH, nh], f32)
            nc.sync.dma_start(va_f[:], vaT[:])
            ones = const.tile([P, P], f16)
            nc.any.memset(ones[:], 1.0)

            # tokT: feature-major tokens [p, k, tok, b]; later reused for attT
            tokT = ptok.tile([P, 3, kt, b_c], f16)

            def transpose_block(src_ap, dst_ap):
                """PE-transpose a [128, 128] block; evict (cast) on DVE."""
                tp = ps_tp.tile([P, P], f32, tag="tp")
                nc.tensor.transpose(tp[:], src_ap, ident[:])
                nc.vector.tensor_copy(dst_ap, tp[:])

            # ---- phase 1: XBAR-transposed token loads (feature-major) ----
            origFM = dram.tile([P, 3, kt, b_c], f16)
            for t in (2, 1, 0):
                nc.sync.dma_start_transpose(tokT[:, t, :, :], tok_in[t][:])
                # stash pristine feature-major token for the phase-5 residual
                nc.sync.dma_start(origFM[:, t, :, :], tokT[:, t, :, :])

            def load_wT(Wt, row_tile):
                """XBAR-transposed load of fp16 weight rows [128, d] into the
                W.T block [128 (in-feature part), kt, 128 (out cols)]."""
                wT = pwT.tile([P, kt, P], f16, tag="wT")
                nc.sync.dma_start_transpose(
                    wT[:], Wt[row_tile * P : (row_tile + 1) * P, :]
                )
                return wT

            # ---- phases 2+3: gated fusion ----
            # G_L = sigmoid(fdbk @ WgL.T + bgL); lat *= G_L
            # G_X = sigmoid(lat' @ WgX.T + bgX); x *= G_X
            for stage, (Wg, bg_t, src_tok, dst_tok) in enumerate(
                [(WgL, bgl_t, 2, 1), (WgX, bgx_t, 1, 0)]
            ):
                for ot in range(kt):
                    wT = load_wT(Wg, ot)
                    pg = ps_mm.tile([P, b_c], f32, tag="mm")
                    for k in range(kt):
                        nc.tensor.matmul(
                            pg[:],
                            wT[:, k, :],
                            tokT[:, src_tok, k, :],
                            start=(k == 0),
                            stop=(k == kt - 1),
                        )
                    gate = pact.tile([P, b_c], f32, tag="gate")
                    nc.scalar.activation(
                        gate[:], pg[:], AF.Sigmoid, bias=bg_t[:, ot : ot + 1]
                    )
                    nc.vector.tensor_mul(
                        tokT[:, dst_tok, ot, :],
                        tokT[:, dst_tok, ot, :],
                        gate[:],
                    )

            # ---- phase 4: per-head QKV + additive attention ----
            attD = [dram.tile([P, 3, b_c], f16, name=f"attD{i}") for i in range(nh)]
            for hh in range(nh):
                qkv_sb = []
                for Wp in (Wq, Wk, Wv):
                    wT = load_wT(Wp, hh)
                    pool_p = pvh if len(qkv_sb) == 2 else pact1
                    dst = pool_p.tile([P, 3, b_c], f16, tag=f"p{len(qkv_sb)}")
                    pps = [ps_mm.tile([P, b_c], f32, tag="mm", name=f"pp{t}") for t in range(3)]
                    for k in range(kt):
                        for t in range(3):
                            nc.tensor.matmul(
                                pps[t][:],
                                wT[:, k, :],
                                tokT[:, t, k, :],
                                start=(k == 0),
                                stop=(k == kt - 1),
                            )
                    for t in range(3):
                        nc.any.tensor_copy(dst[:, t, :], pps[t][:])
                    qkv_sb.append(dst)
                qh, kh, vh = qkv_sb

                # T = tanh(q_i + k_j), bf16 [p, ij, b]
                Tt = pact1.tile([P, 9, b_c], f16, tag="Tt")
                for i in range(3):
                    for j in range(3):
                        pre = pact.tile([P, b_c], f16, tag="Tpre")
                        nc.vector.tensor_add(pre[:], qh[:, i, :], kh[:, j, :])
                        nc.scalar.activation(Tt[:, 3 * i + j, :], pre[:], AF.Tanh)

                # scores: tva = Tt * va_h (per-partition scalar), then
                # column-sum via an all-ones stationary matmul -> every psum
                # partition row holds the scores for 512 b (pre-broadcast).
                tva = pact1.tile([P, 9, b_c], f16, tag="tva")
                nc.vector.tensor_scalar_mul(tva[:], Tt[:], va_f[:, hh : hh + 1])
                attS = pact1.tile([P, 3, b_c], f16, tag="attS")
                for i in range(3):
                    # unnormalized softmax-weighted sum, single normalize at end
                    Ej = []
                    for j in range(3):
                        sc = ps_mm.tile([P, b_c], f32, tag="mm", name=f"sc{j}")
                        nc.tensor.matmul(
                            sc[:],
                            ones[:],
                            tva[:, 3 * i + j, :],
                            start=True,
                            stop=True,
                        )
                        e = psm.tile([P, b_c], f16, tag=f"E{j}")
                        nc.scalar.activation(e[:], sc[:], AF.Exp)
                        Ej.append(e)
                    den = psm.tile([P, b_c], f32, tag="den")
                    nc.vector.tensor_add(den[:], Ej[0][:], Ej[1][:])
                    nc.vector.tensor_add(den[:], den[:], Ej[2][:])
                    rden = psm.tile([P, b_c], f32, tag="rden")
                    nc.vector.reciprocal(rden[:], den[:])
                    acc = pact.tile([P, b_c], f32, tag="acc")
                    tmp = pact.tile([P, b_c], f32, tag="tmp")
                    nc.vector.tensor_mul(acc[:], vh[:, 0, :], Ej[0][:])
                    nc.vector.tensor_mul(tmp[:], vh[:, 1, :], Ej[1][:])
                    nc.vector.tensor_add(acc[:], acc[:], tmp[:])
                    nc.vector.tensor_mul(tmp[:], vh[:, 2, :], Ej[2][:])
                    nc.vector.tensor_add(acc[:], acc[:], tmp[:])
                    nc.vector.tensor_mul(attS[:, i, :], acc[:], rden[:])
                nc.sync.dma_start(attD[hh][:], attS[:])

            # ---- phase 5: output projection + residual ----
            # reuse tokT space for attT (same shape/layout, i = h*128 + d)
            for k in range(kt):
                nc.sync.dma_start(tokT[:, :, k, :], attD[k][:])
            for ot in range(kt):
                wT = load_wT(Wo, ot)
                for t in range(3):
                    po = ps_mm.tile([P, b_c], f32, tag="mm")
                    for k in range(kt):
                        nc.tensor.matmul(
                            po[:],
                            wT[:, k, :],
                            tokT[:, t, k, :],
                            start=(k == 0),
                            stop=(k == kt - 1),
                        )
                    # residual add in feature-major, then transpose out
                    origF = pact.tile([P, b_c], f16, tag="origF")
                    nc.sync.dma_start(origF[:], origFM[:, t, ot, :])
                    oTs = pact.tile([P, b_c], f32, tag="oTs")
                    nc.vector.tensor_add(oTs[:], po[:], origF[:])
                    obm3 = pact.tile([P, nb, P], f32, tag="obm3")
                    for bt in range(nb):
                        tp = ps_tp.tile([P, P], f32, tag="tp")
                        nc.tensor.transpose(
                            tp[:], oTs[:, bt * P : (bt + 1) * P], ident[:]
                        )
                        nc.any.tensor_copy(obm3[:, bt, :], tp[:])
                    nc.sync.dma_start(
                        outs[t][:, ot * P : (ot + 1) * P].rearrange(
                            "(bt p) o -> p bt o", p=P
                        ),
                        obm3[:],
                    )

    nc.compile()
    return nc


def _get_nc():
    key = "full"
    if key not in _compiled:
        _compiled[key] = _build()
    return _compiled[key]


def kernel(
    x_token,
    lat_token,
    fdbk_token,
    W_gate_L,
    b_gate_L,
    W_gate_X,
    b_gate_X,
    W_q,
    W_k,
    W_v,
    W_o,
    v_a,
):
    from concourse.bass_utils import run_bass_kernel_spmd

    nc = _get_nc()

    f32 = np.float32
    x2 = np.ascontiguousarray(np.asarray(x_token, f32).reshape(B, D).astype(np.float16))
    l2 = np.ascontiguousarray(np.asarray(lat_token, f32).reshape(B, D).astype(np.float16))
    f2 = np.ascontiguousarray(np.asarray(fdbk_token, f32).reshape(B, D).astype(np.float16))

    f16 = np.float16
    wgl = np.ascontiguousarray(np.asarray(W_gate_L, f32).astype(f16))
    wgx = np.ascontiguousarray(np.asarray(W_gate_X, f32).astype(f16))
    wq = np.ascontiguousarray(np.asarray(W_q, f32).astype(f16))
    wk = np.ascontiguousarray(np.asarray(W_k, f32).astype(f16))
    wv = np.ascontiguousarray(np.asarray(W_v, f32).astype(f16))
    wo = np.ascontiguousarray(np.asarray(W_o, f32).astype(f16))
    bglT = np.ascontiguousarray(np.asarray(b_gate_L, f32).reshape(KT, P).T)
    bgxT = np.ascontiguousarray(np.asarray(b_gate_X, f32).reshape(KT, P).T)
    vaT = np.ascontiguousarray(np.asarray(v_a, f32).reshape(H, DH).T)
    in_maps = []
    for c in range(N_CORES):
        s = slice(c * B_C, (c + 1) * B_C)
        in_maps.append(
            {
                "x": np.ascontiguousarray(x2[s]),
                "lat": np.ascontiguousarray(l2[s]),
                "fdbk": np.ascontiguousarray(f2[s]),
                "WgL": wgl,
                "WgX": wgx,
                "Wq": wq,
                "Wk": wk,
                "Wv": wv,
                "Wo": wo,
                "bgLT": bglT,
                "bgXT": bgxT,
                "vaT": vaT,
            }
        )

    res = run_bass_kernel_spmd(nc, in_maps, list(range(N_CORES))).results

    out = []
    for t in range(3):
        full = np.concatenate([res[c][f"o{t}"] for c in range(N_CORES)], axis=0)
        out.append(full.reshape(B, 1, D))
    return tuple(out)



# revision 4
# speedup vs baseline: 1.7840x; 1.7840x over previous
"""Trainium2 Bass kernel for nn_FLB_Attention_Layer (gated fusion + additive
attention over 3 tokens + output projection, with residuals).

Data-parallel over batch B=4096 across 8 NeuronCores (512 samples/core,
weights replicated). Device computes the attention-layer output (without
residual) in feature-major layout; host adds the residual and transposes
back to batch-major.

Numerics: all six D x D GEMMs run as fp8e4 (e4m3) DoubleRow matmuls
(2 fp8 weights per PE cell, 256-deep contraction per instruction).
Weights are host-prescaled by 16 and pre-transposed to W.T [in, out];
tokens are host-prescaled by 16 and pre-transposed to feature-major
[D, B_C] f16. QKV psums are 256x true scale; activation-engine evictions
fold the rescale into their scale argument. Additive-attention scores are
computed with a per-head broadcast matmul (lhsT = va replicated across
128 columns, x64 scale) so softmax runs on partition-replicated tiles;
the reciprocal uses the fast approx DVE op.
"""

import numpy as np

P = 128
D = 2048
H = 16
DH = 128
KT = D // P  # 16
B = 4096
N_CORES = 8
B_C = B // N_CORES  # 512

SW = 16.0  # weight prescale (host)
ST = 16.0  # token prescale (host)
SV = 64.0  # v_a prescale (host)

_compiled = {}


def _build(b_c=B_C, d=D, h=H):
    import concourse.bass as bass
    import concourse.mybir as mybir
    import concourse.tile as tile
    from contextlib import ExitStack
    from concourse import bacc

    f32 = mybir.dt.float32
    f16 = mybir.dt.float16
    f8 = mybir.dt.float8e4
    AF = mybir.ActivationFunctionType
    DR = mybir.MatmulPerfMode.DoubleRow

    kt = d // P
    nh = h

    nc = bacc.Bacc(None, target_bir_lowering=False, debug=False)

    toks = [
        nc.declare_dram_parameter(f"tok{t}", [d, b_c], f16, isOutput=False)
        for t in range(2)
    ]
    fdbk8 = nc.declare_dram_parameter("fdbk8", [d, b_c], f8, isOutput=False)
    wgl = nc.declare_dram_parameter("wgl", [d, d], f8, isOutput=False)
    wgx = nc.declare_dram_parameter("wgx", [d, d], f8, isOutput=False)
    wq = nc.declare_dram_parameter("wq", [d, d], f8, isOutput=False)
    wk = nc.declare_dram_parameter("wk", [d, d], f8, isOutput=False)
    wv = nc.declare_dram_parameter("wv", [d, d], f8, isOutput=False)
    wo = nc.declare_dram_parameter("wo", [d, d], f8, isOutput=False)
    bglT = nc.declare_dram_parameter("bglT", [P, kt], f32, isOutput=False)
    bgxT = nc.declare_dram_parameter("bgxT", [P, kt], f32, isOutput=False)
    vab = nc.declare_dram_parameter("vab", [P, nh * P], f8, isOutput=False)
    outs = [
        nc.declare_dram_parameter(f"o{t}", [d, b_c], f16, isOutput=True)
        for t in range(3)
    ]

    with tile.TileContext(nc) as tc:
        with ExitStack() as ctx:
            const = ctx.enter_context(tc.tile_pool(name="const", bufs=1))
            ptok = ctx.enter_context(tc.tile_pool(name="ptok", bufs=1))
            pw = ctx.enter_context(tc.tile_pool(name="pw", bufs=3))
            pg = ctx.enter_context(tc.tile_pool(name="pg", bufs=2))
            pqk = ctx.enter_context(tc.tile_pool(name="pqk", bufs=2))
            pv = ctx.enter_context(tc.tile_pool(name="pv", bufs=3))
            pT = ctx.enter_context(tc.tile_pool(name="pT", bufs=2))
            pE = ctx.enter_context(tc.tile_pool(name="pE", bufs=2))
            psm = ctx.enter_context(tc.tile_pool(name="psm", bufs=1))
            pout = ctx.enter_context(tc.tile_pool(name="pout", bufs=3))
            ps_mm = ctx.enter_context(tc.tile_pool(name="ps_mm", bufs=4, space="PSUM"))
            ps_s = ctx.enter_context(tc.tile_pool(name="ps_s", bufs=3, space="PSUM"))

            bgl_t = const.tile([P, kt], f32)
            bgx_t = const.tile([P, kt], f32)
            va8 = const.tile([P, nh, P], f8)
            nc.sync.dma_start(bgl_t[:], bglT[:])
            nc.sync.dma_start(bgx_t[:], bgxT[:])
            nc.sync.dma_start(va8[:], vab[:])

            # master x/lat tokens (16x true, f16) and fp8 mirror for matmul rhs
            tokF = ptok.tile([P, kt, 2, b_c], f16)
            tok8 = ptok.tile([P, kt, 3, b_c], f8)
            attT = ptok.tile([P, kt, 3, b_c], f8)
            for t in range(2):
                nc.sync.dma_start(
                    tokF[:, :, t, :],
                    toks[t][:].rearrange("(k p) b -> p k b", p=P),
                )
            nc.sync.dma_start(
                tok8[:, :, 2, :], fdbk8[:].rearrange("(k p) b -> p k b", p=P)
            )

            def load_w(Wd, half):
                wh = pw.tile([P, kt, 1024], f8, tag="wh")
                nc.sync.dma_start(
                    wh[:],
                    Wd[:, half * 1024 : (half + 1) * 1024].rearrange(
                        "(k p) o -> p k o", p=P
                    ),
                )
                return wh

            def gemm_chain(psum, wh, oc, rhs_kpt):
                """8 DoubleRow matmuls contracting all of D."""
                for kp in range(8):
                    nc.tensor.matmul(
                        psum[:],
                        wh[:, 2 * kp : 2 * kp + 2, oc * P : (oc + 1) * P],
                        rhs_kpt[:, 2 * kp : 2 * kp + 2, :],
                        start=(kp == 0),
                        stop=(kp == 7),
                        perf_mode=DR,
                    )

            # ---- gated fusion ----
            # G_L = sigmoid(fdbk @ WgL.T + bgL); lat' = lat * G_L
            # G_X = sigmoid(lat' @ WgX.T + bgX); x'   = x * G_X
            for Wd, bg_t, src_t, dst_t in ((wgl, bgl_t, 2, 1), (wgx, bgx_t, 1, 0)):
                for half in range(2):
                    wh = load_w(Wd, half)
                    for oc in range(8):
                        ot = half * 8 + oc
                        pgp = ps_mm.tile([P, b_c], f32, tag="mm")
                        gemm_chain(pgp, wh, oc, tok8[:, :, src_t, :])
                        gate = pg.tile([P, b_c], f16, tag="gate")
                        nc.scalar.activation(
                            gate[:],
                            pgp[:],
                            AF.Sigmoid,
                            bias=bg_t[:, ot : ot + 1],
                            scale=1.0 / (SW * ST),
                        )
                        nc.vector.tensor_mul(
                            tok8[:, ot, dst_t, :],
                            tokF[:, ot, dst_t, :],
                            gate[:],
                        )

            # ---- QKV + additive attention, 2 groups of 8 heads ----
            def emit_attn(hh, T8, vh):
                for i in range(3):
                    E = pE.tile([P, 3, b_c], f16, tag="E")
                    for j in range(3):
                        sps = ps_s.tile([P, b_c], f32, tag="sc")
                        nc.tensor.matmul(
                            sps[:],
                            va8[:, hh, :],
                            T8[:, 3 * i + j, :],
                            start=True,
                            stop=True,
                        )
                        nc.scalar.activation(
                            E[:, j, :], sps[:], AF.Exp, scale=1.0 / SV
                        )
                    den = psm.tile([P, b_c], f32, tag="den")
                    nc.vector.tensor_add(den[:], E[:, 0, :], E[:, 1, :])
                    nc.vector.tensor_add(den[:], den[:], E[:, 2, :])
                    rden = psm.tile([P, b_c], f32, tag="rden")
                    nc.vector.reciprocal_approx_fast(rden[:], den[:])
                    acc = psm.tile([P, b_c], f32, tag="acc")
                    tmp = psm.tile([P, b_c], f32, tag="tmp")
                    nc.vector.tensor_mul(acc[:], E[:, 0, :], vh[:, 0, :])
                    nc.vector.tensor_mul(tmp[:], E[:, 1, :], vh[:, 1, :])
                    nc.vector.tensor_add(acc[:], acc[:], tmp[:])
                    nc.vector.tensor_mul(tmp[:], E[:, 2, :], vh[:, 2, :])
                    nc.vector.tensor_add(acc[:], acc[:], tmp[:])
                    nc.vector.tensor_mul(attT[:, hh, i, :], acc[:], rden[:])

            pend = None
            for g in range(2):
                whq = load_w(wq, g)
                whk = load_w(wk, g)
                whv = load_w(wv, g)
                for hg in range(8):
                    hh = g * 8 + hg
                    qf = pqk.tile([P, 3, b_c], f16, tag="qf")
                    kf = pqk.tile([P, 3, b_c], f16, tag="kf")
                    vh = pv.tile([P, 3, b_c], f16, tag="vh")
                    for wh_, dst in ((whq, qf), (whk, kf), (whv, vh)):
                        for t in range(3):
                            pp = ps_mm.tile([P, b_c], f32, tag="mm")
                            gemm_chain(pp, wh_, hg, tok8[:, :, t, :])
                            nc.scalar.activation(
                                dst[:, t, :], pp[:], AF.Copy, scale=1.0 / SW
                            )
                    T8 = pT.tile([P, 9, b_c], f8, tag="T8")
                    for i in range(3):
                        for j in range(3):
                            pre = pg.tile([P, b_c], f16, tag="pre")
                            nc.vector.tensor_add(
                                pre[:], qf[:, i, :], kf[:, j, :]
                            )
                            nc.scalar.activation(
                                T8[:, 3 * i + j, :], pre[:], AF.Tanh, scale=1.0 / ST
                            )
                    if pend is not None:
                        emit_attn(*pend)
                    pend = (hh, T8, vh)
            emit_attn(*pend)

            # ---- output projection (no residual; host adds it) ----
            for half in range(2):
                who = load_w(wo, half)
                for oc in range(8):
                    ot = half * 8 + oc
                    for t in range(3):
                        po = ps_mm.tile([P, b_c], f32, tag="mm")
                        gemm_chain(po, who, oc, attT[:, :, t, :])
                        of = pout.tile([P, b_c], f16, tag="of")
                        nc.scalar.activation(of[:], po[:], AF.Copy, scale=1.0 / SW)
                        nc.sync.dma_start(
                            outs[t][ot * P : (ot + 1) * P, :], of[:]
                        )

    nc.compile()
    return nc


def _get_nc():
    key = "full"
    if key not in _compiled:
        _compiled[key] = _build()
    return _compiled[key]


def kernel(
    x_token,
    lat_token,
    fdbk_token,
    W_gate_L,
    b_gate_L,
    W_gate_X,
    b_gate_X,
    W_q,
    W_k,
    W_v,
    W_o,
    v_a,
):
    import ml_dtypes
    from concourse.bass_utils import run_bass_kernel_spmd

    nc = _get_nc()
    f32 = np.float32
    e4 = ml_dtypes.float8_e4m3

    def prep_w(W):
        wt = np.asarray(W, f32).T * SW
        return np.ascontiguousarray(np.clip(wt, -240.0, 240.0)).astype(e4)

    w8 = {
        "wgl": prep_w(W_gate_L),
        "wgx": prep_w(W_gate_X),
        "wq": prep_w(W_q),
        "wk": prep_w(W_k),
        "wv": prep_w(W_v),
        "wo": prep_w(W_o),
    }
    bglT = np.ascontiguousarray(np.asarray(b_gate_L, f32).reshape(KT, P).T)
    bgxT = np.ascontiguousarray(np.asarray(b_gate_X, f32).reshape(KT, P).T)
    va = np.asarray(v_a, f32).reshape(H, DH).T * SV  # [DH, H]
    vab = np.ascontiguousarray(
        np.repeat(va[:, :, None], P, axis=2).reshape(DH, H * P)
    ).astype(e4)

    tok_full = [
        np.asarray(t, f32).reshape(B, D) for t in (x_token, lat_token, fdbk_token)
    ]
    tokT16 = [
        np.ascontiguousarray(tok_full[t].T * ST).astype(np.float16) for t in range(2)
    ]
    fdbkT8 = np.clip(tok_full[2].T * ST, -240.0, 240.0).astype(e4)

    in_maps = []
    for c in range(N_CORES):
        s = slice(c * B_C, (c + 1) * B_C)
        m = {f"tok{t}": np.ascontiguousarray(tokT16[t][:, s]) for t in range(2)}
        m["fdbk8"] = np.ascontiguousarray(fdbkT8[:, s])
        m.update(w8)
        m.update({"bglT": bglT, "bgxT": bgxT, "vab": vab})
        in_maps.append(m)

    res = run_bass_kernel_spmd(nc, in_maps, list(range(N_CORES))).results

    out = []
    for t in range(3):
        full = np.concatenate([res[c][f"o{t}"] for c in range(N_CORES)], axis=1)
        o = full.T.astype(f32) / SW + tok_full[t]
        out.append(o.reshape(B, 1, D))
    return tuple(out)


# revision 6
# speedup vs baseline: 1.8893x; 1.0590x over previous
"""Trainium2 Bass kernel for nn_FLB_Attention_Layer (gated fusion + additive
attention over 3 tokens + output projection, with residuals).

Data-parallel over batch B=4096 across 8 NeuronCores (512 samples/core,
weights replicated). Device computes the attention-layer output (without
residual) in feature-major layout; host adds the residual and transposes
back to batch-major.

Numerics: all six D x D GEMMs run as fp8e4 (e4m3) DoubleRow matmuls
(2 fp8 weights per PE cell, 256-deep contraction per instruction).
Weights are host-prescaled by 16 and pre-transposed to W.T [in, out];
tokens are host-prescaled by 16 and pre-transposed to feature-major
[D, B_C] f16. QKV psums are 256x true scale; activation-engine evictions
fold the rescale into their scale argument. Additive-attention scores are
computed with a per-head broadcast matmul (lhsT = va replicated across
128 columns, x64 scale) so softmax runs on partition-replicated tiles;
the reciprocal uses the fast approx DVE op.
"""

import numpy as np

P = 128
D = 2048
H = 16
DH = 128
KT = D // P  # 16
B = 4096
N_CORES = 8
B_C = B // N_CORES  # 512

SW = 16.0  # weight prescale (host)
ST = 16.0  # token prescale (host)
SV = 64.0  # v_a prescale (host)

_compiled = {}


def _build(b_c=B_C, d=D, h=H):
    import concourse.bass as bass
    import concourse.mybir as mybir
    import concourse.tile as tile
    from contextlib import ExitStack
    from concourse import bacc

    f32 = mybir.dt.float32
    f16 = mybir.dt.float16
    f8 = mybir.dt.float8e4
    AF = mybir.ActivationFunctionType
    DR = mybir.MatmulPerfMode.DoubleRow

    kt = d // P
    nh = h

    nc = bacc.Bacc(None, target_bir_lowering=False, debug=False)

    toks = [
        nc.declare_dram_parameter(f"tok{t}", [d, b_c], f16, isOutput=False)
        for t in range(2)
    ]
    fdbk8 = nc.declare_dram_parameter("fdbk8", [d, b_c], f8, isOutput=False)
    wgl = nc.declare_dram_parameter("wgl", [d, d], f8, isOutput=False)
    wgx = nc.declare_dram_parameter("wgx", [d, d], f8, isOutput=False)
    wq = nc.declare_dram_parameter("wq", [d, d], f8, isOutput=False)
    wk = nc.declare_dram_parameter("wk", [d, d], f8, isOutput=False)
    wv = nc.declare_dram_parameter("wv", [d, d], f8, isOutput=False)
    wo = nc.declare_dram_parameter("wo", [d, d], f8, isOutput=False)
    bglT = nc.declare_dram_parameter("bglT", [P, kt], f32, isOutput=False)
    bgxT = nc.declare_dram_parameter("bgxT", [P, kt], f32, isOutput=False)
    vab = nc.declare_dram_parameter("vab", [P, nh * P], f8, isOutput=False)
    outs = [
        nc.declare_dram_parameter(f"o{t}", [d, b_c], f16, isOutput=True)
        for t in range(3)
    ]

    with tile.TileContext(nc) as tc:
        with ExitStack() as ctx:
            const = ctx.enter_context(tc.tile_pool(name="const", bufs=1))
            ptok = ctx.enter_context(tc.tile_pool(name="ptok", bufs=1))
            pw = ctx.enter_context(tc.tile_pool(name="pw", bufs=4))
            pg = ctx.enter_context(tc.tile_pool(name="pg", bufs=2))
            pqk = ctx.enter_context(tc.tile_pool(name="pqk", bufs=2))
            pv = ctx.enter_context(tc.tile_pool(name="pv", bufs=2))
            pT = ctx.enter_context(tc.tile_pool(name="pT", bufs=2))
            pE = ctx.enter_context(tc.tile_pool(name="pE", bufs=2))
            psm = ctx.enter_context(tc.tile_pool(name="psm", bufs=1))
            pout = ctx.enter_context(tc.tile_pool(name="pout", bufs=3))
            ps_mm = ctx.enter_context(tc.tile_pool(name="ps_mm", bufs=2, space="PSUM"))
            ps_s = ctx.enter_context(tc.tile_pool(name="ps_s", bufs=2, space="PSUM"))

            bgl_t = const.tile([P, kt], f32)
            bgx_t = const.tile([P, kt], f32)
            va8 = const.tile([P, nh, P], f8)
            nc.sync.dma_start(bgl_t[:], bglT[:])
            nc.sync.dma_start(bgx_t[:], bgxT[:])
            nc.sync.dma_start(va8[:], vab[:])

            # master x/lat tokens (16x true, f16) and fp8 mirror for matmul rhs
            tokF = ptok.tile([P, kt, 2, b_c], f16)
            tok8 = ptok.tile([P, kt, 3, b_c], f8)
            attT = ptok.tile([P, kt, 3, b_c], f8)
            for t in range(2):
                nc.sync.dma_start(
                    tokF[:, :, t, :],
                    toks[t][:].rearrange("(k p) b -> p k b", p=P),
                )
            nc.sync.dma_start(
                tok8[:, :, 2, :], fdbk8[:].rearrange("(k p) b -> p k b", p=P)
            )

            def load_w(Wd, half):
                wh = pw.tile([P, kt, 1024], f8, tag="wh")
                nc.sync.dma_start(
                    wh[:],
                    Wd[:, half * 1024 : (half + 1) * 1024].rearrange(
                        "(k p) o -> p k o", p=P
                    ),
                )
                return wh

            def gemm_chain(psum_ap, wh, oc, rhs_kpt):
                """8 DoubleRow matmuls contracting all of D."""
                for kp in range(8):
                    nc.tensor.matmul(
                        psum_ap,
                        wh[:, 2 * kp : 2 * kp + 2, oc * P : (oc + 1) * P],
                        rhs_kpt[:, 2 * kp : 2 * kp + 2, :],
                        start=(kp == 0),
                        stop=(kp == 7),
                        perf_mode=DR,
                    )

            # ---- gated fusion ----
            # G_L = sigmoid(fdbk @ WgL.T + bgL); lat' = lat * G_L
            # G_X = sigmoid(lat' @ WgX.T + bgX); x'   = x * G_X
            for Wd, bg_t, src_t, dst_t in ((wgl, bgl_t, 2, 1), (wgx, bgx_t, 1, 0)):
                for half in range(2):
                    wh = load_w(Wd, half)
                    for oc in range(8):
                        ot = half * 8 + oc
                        pgp = ps_mm.tile([P, b_c], f32, tag="mm")
                        gemm_chain(pgp[:], wh, oc, tok8[:, :, src_t, :])
                        gate = pg.tile([P, b_c], f16, tag="gate")
                        nc.scalar.activation(
                            gate[:],
                            pgp[:],
                            AF.Sigmoid,
                            bias=bg_t[:, ot : ot + 1],
                            scale=1.0 / (SW * ST),
                        )
                        nc.vector.tensor_mul(
                            tok8[:, ot, dst_t, :],
                            tokF[:, ot, dst_t, :],
                            gate[:],
                        )

            # ---- QKV + additive attention, 2 groups of 8 heads ----
            def emit_attn(hh, T8, vh):
                for i in range(3):
                    E = pE.tile([P, 3, b_c], f16, tag="E")
                    sps = ps_s.tile([P, 3, b_c], f32, tag="sc")
                    for j in range(3):
                        nc.tensor.matmul(
                            sps[:, j, :],
                            va8[:, hh, :],
                            T8[:, 3 * i + j, :],
                            start=True,
                            stop=True,
                        )
                    nc.scalar.activation(E[:], sps[:], AF.Exp, scale=1.0 / SV)
                    den = psm.tile([P, b_c], f32, tag="den")
                    nc.vector.tensor_add(den[:], E[:, 0, :], E[:, 1, :])
                    nc.vector.tensor_add(den[:], den[:], E[:, 2, :])
                    rden = psm.tile([P, b_c], f32, tag="rden")
                    nc.vector.reciprocal_approx_fast(rden[:], den[:])
                    acc = psm.tile([P, b_c], f32, tag="acc")
                    tmp = psm.tile([P, b_c], f32, tag="tmp")
                    nc.vector.tensor_mul(acc[:], E[:, 0, :], vh[:, 0, :])
                    nc.vector.tensor_mul(tmp[:], E[:, 1, :], vh[:, 1, :])
                    nc.vector.tensor_add(acc[:], acc[:], tmp[:])
                    nc.vector.tensor_mul(tmp[:], E[:, 2, :], vh[:, 2, :])
                    nc.vector.tensor_add(acc[:], acc[:], tmp[:])
                    nc.vector.tensor_mul(attT[:, hh, i, :], acc[:], rden[:])

            pend = None
            for g in range(2):
                whq = load_w(wq, g)
                whk = load_w(wk, g)
                whv = load_w(wv, g)
                for hg in range(8):
                    hh = g * 8 + hg
                    qf = pqk.tile([P, 3, b_c], f16, tag="qf")
                    kf = pqk.tile([P, 3, b_c], f16, tag="kf")
                    vh = pv.tile([P, 3, b_c], f16, tag="vh")
                    for wh_, dst in ((whq, qf), (whk, kf), (whv, vh)):
                        for t in range(3):
                            pp = ps_mm.tile([P, b_c], f32, tag="mm")
                            gemm_chain(pp[:], wh_, hg, tok8[:, :, t, :])
                            nc.scalar.activation(
                                dst[:, t, :], pp[:], AF.Copy, scale=1.0 / SW
                            )
                    T8 = pT.tile([P, 9, b_c], f8, tag="T8")
                    for i in range(3):
                        pre3 = pg.tile([P, 3, b_c], f16, tag="pre")
                        for j in range(3):
                            nc.vector.tensor_add(
                                pre3[:, j, :], qf[:, i, :], kf[:, j, :]
                            )
                        nc.scalar.activation(
                            T8[:, 3 * i : 3 * i + 3, :], pre3[:], AF.Tanh,
                            scale=1.0 / ST,
                        )
                    if pend is not None:
                        emit_attn(*pend)
                    pend = (hh, T8, vh)
            emit_attn(*pend)

            # ---- output projection (no residual; host adds it) ----
            for half in range(2):
                who = load_w(wo, half)
                for oc in range(8):
                    ot = half * 8 + oc
                    po3 = ps_s.tile([P, 3, b_c], f32, tag="sc")
                    for t in range(3):
                        gemm_chain(po3[:, t, :], who, oc, attT[:, :, t, :])
                    of3 = pout.tile([P, 3, b_c], f16, tag="of")
                    nc.scalar.activation(of3[:], po3[:], AF.Copy, scale=1.0 / SW)
                    for t in range(3):
                        nc.sync.dma_start(
                            outs[t][ot * P : (ot + 1) * P, :], of3[:, t, :]
                        )

    nc.compile()
    return nc


def _get_nc():
    key = "full"
    if key not in _compiled:
        _compiled[key] = _build()
    return _compiled[key]


def kernel(
    x_token,
    lat_token,
    fdbk_token,
    W_gate_L,
    b_gate_L,
    W_gate_X,
    b_gate_X,
    W_q,
    W_k,
    W_v,
    W_o,
    v_a,
):
    import ml_dtypes
    from concourse.bass_utils import run_bass_kernel_spmd

    nc = _get_nc()
    f32 = np.float32
    e4 = ml_dtypes.float8_e4m3

    def prep_w(W):
        wt = np.asarray(W, f32).T * SW
        return np.ascontiguousarray(np.clip(wt, -240.0, 240.0)).astype(e4)

    w8 = {
        "wgl": prep_w(W_gate_L),
        "wgx": prep_w(W_gate_X),
        "wq": prep_w(W_q),
        "wk": prep_w(W_k),
        "wv": prep_w(W_v),
        "wo": prep_w(W_o),
    }
    bglT = np.ascontiguousarray(np.asarray(b_gate_L, f32).reshape(KT, P).T)
    bgxT = np.ascontiguousarray(np.asarray(b_gate_X, f32).reshape(KT, P).T)
    va = np.asarray(v_a, f32).reshape(H, DH).T * SV  # [DH, H]
    vab = np.ascontiguousarray(
        np.repeat(va[:, :, None], P, axis=2).reshape(DH, H * P)
    ).astype(e4)

    tok_full = [
        np.asarray(t, f32).reshape(B, D) for t in (x_token, lat_token, fdbk_token)
    ]
    tokT16 = [
        np.ascontiguousarray(tok_full[t].T * ST).astype(np.float16) for t in range(2)
    ]
    fdbkT8 = np.clip(tok_full[2].T * ST, -240.0, 240.0).astype(e4)

    in_maps = []
    for c in range(N_CORES):
        s = slice(c * B_C, (c + 1) * B_C)
        m = {f"tok{t}": np.ascontiguousarray(tokT16[t][:, s]) for t in range(2)}
        m["fdbk8"] = np.ascontiguousarray(fdbkT8[:, s])
        m.update(w8)
        m.update({"bglT": bglT, "bgxT": bgxT, "vab": vab})
        in_maps.append(m)

    res = run_bass_kernel_spmd(nc, in_maps, list(range(N_CORES))).results

    out = []
    for t in range(3):
        full = np.concatenate([res[c][f"o{t}"] for c in range(N_CORES)], axis=1)
        o = full.T.astype(f32) / SW + tok_full[t]
        out.append(o.reshape(B, 1, D))
    return tuple(out)


# revision 7
# speedup vs baseline: 2.0640x; 1.0924x over previous
"""Trainium2 Bass kernel for nn_FLB_Attention_Layer (gated fusion + additive
attention over 3 tokens + output projection, with residuals).

Data-parallel over batch B=4096 across 8 NeuronCores (512 samples/core,
weights replicated). Device computes the attention-layer output (without
residual) in feature-major layout; host adds the residual and transposes
back to batch-major.

Numerics: all six D x D GEMMs run as fp8e4 (e4m3) DoubleRow matmuls
(2 fp8 weights per PE cell, 256-deep contraction per instruction).
Weights are host-prescaled by 16 and pre-transposed to W.T [in, out];
tokens are host-prescaled by 16 and pre-transposed to feature-major
[D, B_C] f16. QKV psums are 256x true scale; activation-engine evictions
fold the rescale into their scale argument. Additive-attention scores are
computed with a per-head broadcast matmul (lhsT = va replicated across
128 columns, x64 scale) so softmax runs on partition-replicated tiles;
the reciprocal uses the fast approx DVE op.
"""

import numpy as np

P = 128
D = 2048
H = 16
DH = 128
KT = D // P  # 16
B = 4096
N_CORES = 8
B_C = B // N_CORES  # 512

SW = 16.0  # weight prescale (host)
ST = 16.0  # token prescale (host)
SV = 64.0  # v_a prescale (host)

_compiled = {}


def _build(b_c=B_C, d=D, h=H):
    import concourse.bass as bass
    import concourse.mybir as mybir
    import concourse.tile as tile
    from contextlib import ExitStack
    from concourse import bacc

    f32 = mybir.dt.float32
    f16 = mybir.dt.float16
    f8 = mybir.dt.float8e4
    AF = mybir.ActivationFunctionType
    DR = mybir.MatmulPerfMode.DoubleRow

    kt = d // P
    nh = h

    nc = bacc.Bacc(None, target_bir_lowering=False, debug=False)

    toks = [
        nc.declare_dram_parameter(f"tok{t}", [d, b_c], f16, isOutput=False)
        for t in range(2)
    ]
    fdbk8 = nc.declare_dram_parameter("fdbk8", [d, b_c], f8, isOutput=False)
    wgl = nc.declare_dram_parameter("wgl", [d, d], f8, isOutput=False)
    wgx = nc.declare_dram_parameter("wgx", [d, d], f8, isOutput=False)
    wq = nc.declare_dram_parameter("wq", [d, d], f8, isOutput=False)
    wk = nc.declare_dram_parameter("wk", [d, d], f8, isOutput=False)
    wv = nc.declare_dram_parameter("wv", [d, d], f8, isOutput=False)
    wo = nc.declare_dram_parameter("wo", [d, d], f8, isOutput=False)
    bglT = nc.declare_dram_parameter("bglT", [P, kt], f32, isOutput=False)
    bgxT = nc.declare_dram_parameter("bgxT", [P, kt], f32, isOutput=False)
    vab = nc.declare_dram_parameter("vab", [P, nh * P], f8, isOutput=False)
    outs = [
        nc.declare_dram_parameter(f"o{t}", [d, b_c], f16, isOutput=True)
        for t in range(3)
    ]

    with tile.TileContext(nc) as tc:
        with ExitStack() as ctx:
            const = ctx.enter_context(tc.tile_pool(name="const", bufs=1))
            ptok = ctx.enter_context(tc.tile_pool(name="ptok", bufs=1))
            pw = ctx.enter_context(tc.tile_pool(name="pw", bufs=4))
            pg = ctx.enter_context(tc.tile_pool(name="pg", bufs=2))
            pqk = ctx.enter_context(tc.tile_pool(name="pqk", bufs=2))
            pv = ctx.enter_context(tc.tile_pool(name="pv", bufs=2))
            pT = ctx.enter_context(tc.tile_pool(name="pT", bufs=2))
            pE = ctx.enter_context(tc.tile_pool(name="pE", bufs=3))
            psm = ctx.enter_context(tc.tile_pool(name="psm", bufs=1))
            pout = ctx.enter_context(tc.tile_pool(name="pout", bufs=3))
            ps_mm = ctx.enter_context(tc.tile_pool(name="ps_mm", bufs=2, space="PSUM"))
            ps_s = ctx.enter_context(tc.tile_pool(name="ps_s", bufs=2, space="PSUM"))

            bgl_t = const.tile([P, kt], f32)
            bgx_t = const.tile([P, kt], f32)
            va8 = const.tile([P, nh, P], f8)
            nc.sync.dma_start(bgl_t[:], bglT[:])
            nc.sync.dma_start(bgx_t[:], bgxT[:])
            nc.sync.dma_start(va8[:], vab[:])

            # master x/lat tokens (16x true, f16) and fp8 mirror for matmul rhs
            tokF = ptok.tile([P, kt, 2, b_c], f16)
            tok8 = ptok.tile([P, kt, 3, b_c], f8)
            attT = ptok.tile([P, kt, 3, b_c], f8)
            nc.scalar.dma_start(
                tok8[:, :, 2, :], fdbk8[:].rearrange("(k p) b -> p k b", p=P)
            )
            for t in (1, 0):
                nc.scalar.dma_start(
                    tokF[:, :, t, :],
                    toks[t][:].rearrange("(k p) b -> p k b", p=P),
                )

            def load_w(Wd, half):
                wh = pw.tile([P, kt, 1024], f8, tag="wh")
                nc.sync.dma_start(
                    wh[:],
                    Wd[:, half * 1024 : (half + 1) * 1024].rearrange(
                        "(k p) o -> p k o", p=P
                    ),
                )
                return wh

            def gemm_chain(psum_ap, wh, oc, rhs_kpt):
                """8 DoubleRow matmuls contracting all of D."""
                for kp in range(8):
                    nc.tensor.matmul(
                        psum_ap,
                        wh[:, 2 * kp : 2 * kp + 2, oc * P : (oc + 1) * P],
                        rhs_kpt[:, 2 * kp : 2 * kp + 2, :],
                        start=(kp == 0),
                        stop=(kp == 7),
                        perf_mode=DR,
                    )

            # ---- gated fusion ----
            # G_L = sigmoid(fdbk @ WgL.T + bgL); lat' = lat * G_L
            # G_X = sigmoid(lat' @ WgX.T + bgX); x'   = x * G_X
            for Wd, bg_t, src_t, dst_t in ((wgl, bgl_t, 2, 1), (wgx, bgx_t, 1, 0)):
                for half in range(2):
                    wh = load_w(Wd, half)
                    for oc in range(8):
                        ot = half * 8 + oc
                        pgp = ps_mm.tile([P, b_c], f32, tag="mm")
                        gemm_chain(pgp[:], wh, oc, tok8[:, :, src_t, :])
                        gate = pg.tile([P, b_c], f16, tag="gate")
                        nc.scalar.activation(
                            gate[:],
                            pgp[:],
                            AF.Sigmoid,
                            bias=bg_t[:, ot : ot + 1],
                            scale=1.0 / (SW * ST),
                        )
                        nc.vector.tensor_mul(
                            tok8[:, ot, dst_t, :],
                            tokF[:, ot, dst_t, :],
                            gate[:],
                        )

            # ---- QKV + additive attention, 2 groups of 8 heads ----
            def emit_scores(hh, T8):
                Es = []
                for i in range(3):
                    E = pE.tile([P, 3, b_c], f16, tag="E", name=f"E{i}")
                    sps = ps_s.tile([P, 3, b_c], f32, tag="sc")
                    for j in range(3):
                        nc.tensor.matmul(
                            sps[:, j, :],
                            va8[:, hh, :],
                            T8[:, 3 * i + j, :],
                            start=True,
                            stop=True,
                        )
                    nc.scalar.activation(E[:], sps[:], AF.Exp, scale=1.0 / SV)
                    Es.append(E)
                return Es

            def emit_softmax(hh, Es, vh):
                for i in range(3):
                    E = Es[i]
                    den = psm.tile([P, b_c], f32, tag="den")
                    nc.vector.tensor_add(den[:], E[:, 0, :], E[:, 1, :])
                    nc.vector.tensor_add(den[:], den[:], E[:, 2, :])
                    rden = psm.tile([P, b_c], f32, tag="rden")
                    nc.vector.reciprocal_approx_fast(rden[:], den[:])
                    acc = psm.tile([P, b_c], f16, tag="acc")
                    tmp = psm.tile([P, b_c], f16, tag="tmp")
                    nc.vector.tensor_mul(acc[:], E[:, 0, :], vh[:, 0, :])
                    nc.vector.tensor_mul(tmp[:], E[:, 1, :], vh[:, 1, :])
                    nc.vector.tensor_add(acc[:], acc[:], tmp[:])
                    nc.vector.tensor_mul(tmp[:], E[:, 2, :], vh[:, 2, :])
                    nc.vector.tensor_add(acc[:], acc[:], tmp[:])
                    nc.vector.tensor_mul(attT[:, hh, i, :], acc[:], rden[:])

            pend = None
            for g in range(2):
                whq = load_w(wq, g)
                whk = load_w(wk, g)
                whv = load_w(wv, g)
                for hg in range(8):
                    hh = g * 8 + hg
                    qf = pqk.tile([P, 3, b_c], f16, tag="qf")
                    kf = pqk.tile([P, 3, b_c], f16, tag="kf")
                    vh = pv.tile([P, 3, b_c], f16, tag="vh")
                    for wh_, dst in ((whq, qf), (whk, kf), (whv, vh)):
                        for t in range(3):
                            pp = ps_mm.tile([P, b_c], f32, tag="mm")
                            gemm_chain(pp[:], wh_, hg, tok8[:, :, t, :])
                            nc.scalar.activation(
                                dst[:, t, :], pp[:], AF.Copy, scale=1.0 / SW
                            )
                    if pend is not None:
                        ph, pT8, pvh = pend
                        pEs = emit_scores(ph, pT8)
                    T8 = pT.tile([P, 9, b_c], f8, tag="T8")
                    for i in range(3):
                        pre3 = pg.tile([P, 3, b_c], f16, tag="pre")
                        for j in range(3):
                            nc.vector.tensor_add(
                                pre3[:, j, :], qf[:, i, :], kf[:, j, :]
                            )
                        nc.scalar.activation(
                            T8[:, 3 * i : 3 * i + 3, :], pre3[:], AF.Tanh,
                            scale=1.0 / ST,
                        )
                    if pend is not None:
                        emit_softmax(ph, pEs, pvh)
                    pend = (hh, T8, vh)
            ph, pT8, pvh = pend
            emit_softmax(ph, emit_scores(ph, pT8), pvh)

            # ---- output projection (no residual; host adds it) ----
            for half in range(2):
                who = load_w(wo, half)
                for oc in range(8):
                    ot = half * 8 + oc
                    po3 = ps_s.tile([P, 3, b_c], f32, tag="sc")
                    for t in range(3):
                        gemm_chain(po3[:, t, :], who, oc, attT[:, :, t, :])
                    of3 = pout.tile([P, 3, b_c], f16, tag="of")
                    nc.scalar.activation(of3[:], po3[:], AF.Copy, scale=1.0 / SW)
                    for t in range(3):
                        nc.sync.dma_start(
                            outs[t][ot * P : (ot + 1) * P, :], of3[:, t, :]
                        )

    nc.compile()
    return nc


def _get_nc():
    key = "full"
    if key not in _compiled:
        _compiled[key] = _build()
    return _compiled[key]


def kernel(
    x_token,
    lat_token,
    fdbk_token,
    W_gate_L,
    b_gate_L,
    W_gate_X,
    b_gate_X,
    W_q,
    W_k,
    W_v,
    W_o,
    v_a,
):
    import ml_dtypes
    from concourse.bass_utils import run_bass_kernel_spmd

    nc = _get_nc()
    f32 = np.float32
    e4 = ml_dtypes.float8_e4m3

    def prep_w(W):
        wt = np.asarray(W, f32).T * SW
        return np.ascontiguousarray(np.clip(wt, -240.0, 240.0)).astype(e4)

    w8 = {
        "wgl": prep_w(W_gate_L),
        "wgx": prep_w(W_gate_X),
        "wq": prep_w(W_q),
        "wk": prep_w(W_k),
        "wv": prep_w(W_v),
        "wo": prep_w(W_o),
    }
    bglT = np.ascontiguousarray(np.asarray(b_gate_L, f32).reshape(KT, P).T)
    bgxT = np.ascontiguousarray(np.asarray(b_gate_X, f32).reshape(KT, P).T)
    va = np.asarray(v_a, f32).reshape(H, DH).T * SV  # [DH, H]
    vab = np.ascontiguousarray(
        np.repeat(va[:, :, None], P, axis=2).reshape(DH, H * P)
    ).astype(e4)

    tok_full = [
        np.asarray(t, f32).reshape(B, D) for t in (x_token, lat_token, fdbk_token)
    ]
    tokT16 = [
        np.ascontiguousarray(tok_full[t].T * ST).astype(np.float16) for t in range(2)
    ]
    fdbkT8 = np.clip(tok_full[2].T * ST, -240.0, 240.0).astype(e4)

    in_maps = []
    for c in range(N_CORES):
        s = slice(c * B_C, (c + 1) * B_C)
        m = {f"tok{t}": np.ascontiguousarray(tokT16[t][:, s]) for t in range(2)}
        m["fdbk8"] = np.ascontiguousarray(fdbkT8[:, s])
        m.update(w8)
        m.update({"bglT": bglT, "bgxT": bgxT, "vab": vab})
        in_maps.append(m)

    res = run_bass_kernel_spmd(nc, in_maps, list(range(N_CORES))).results

    out = []
    for t in range(3):
        full = np.concatenate([res[c][f"o{t}"] for c in range(N_CORES)], axis=1)
        o = full.T.astype(f32) / SW + tok_full[t]
        out.append(o.reshape(B, 1, D))
    return tuple(out)
